# revision 9
# baseline (speedup 1.0000x reference)
"""Trainium2 Bass kernel for nn_KP_Decoder (AFT-style decoder + softmax).

Shards data-parallel over batch B across 8 NeuronCores (8 batches/core).
Per batch b on-device:
  k|v   = en[b] @ [Wk.T | Wv.T]           (float32r matmuls, N=256)
  ek    = exp(k); ekv = ek*v              (ACT exp -> f32r, DVE mul -> f32r)
  q     = cat(egmp,cap) @ Wq.T            (accumulating matmuls K=128 + K=1)
  sigq' = sigmoid(q) / (sqrt(D)*c2)       (ACT exp + DVE tensor_scalar + recip)
  eaT   = exp(c1 * cur.T)                 (ACT exp, scale folded)
  den|b = eaT.T-chunks @ [ek|ekv]         (16 accumulating f32r matmuls, N=256)
  aft'  = sigq' * bias / denom            (recip_approx_fast + muls)
  aftT  = transpose(aft')                 (PE transpose)
  s'    = aftT.T @ enT                    (f32r matmuls, N=500; 1/(sqrt(D)c2) folded)
  t     = tanh(c2*(s' + cur))             (DVE add + ACT tanh, c2 as scale)
  e     = exp(CLIP*t), rowsum             (ACT exp + accum_out)
  probs = e * (1/rowsum)                  (DVE reciprocal + tensor_scalar)
"""
import sys
if '/opt/trn_rl_repo' not in sys.path:
    sys.path.insert(0, '/opt/trn_rl_repo')

import numpy as np

B, P, N, D = 64, 200, 2000, 128
SQRT_D = 11.313708498984761
CLIP = 10.0
N_CORES = 8
BPC = B // N_CORES            # batches per core
NCH = 16                      # n-chunks
CHK = N // NCH                # 125 rows per contraction chunk
PCH = P // 2                  # 100, two p-chunks
SCORE_BLK = 1000              # score psum block (2 banks)
SCORE_MM = 500                # free dim per score matmul (<=512 fp32)

_CACHE = {}


def _build(has_mask: bool, repeat: int = 1):
    import concourse.bacc as bacc
    import concourse.mybir as mybir
    import concourse.tile as tile
    from concourse.masks import make_identity

    F32 = mybir.dt.float32
    F32R = mybir.dt.float32r
    AF = mybir.ActivationFunctionType
    ALU = mybir.AluOpType

    nc = bacc.Bacc("TRN2", target_bir_lowering=False, debug=False,
                   num_devices=N_CORES)

    # ---- DRAM I/O (per-core shapes) ----
    # packed per-batch load: [0:N]=enT, [N:N+NCH*P]=curT(rows<CHK), then egT, cap(row0)
    BIGW = N + NCH * P + P + P
    big_d = nc.dram_tensor("big", [BPC, 128, BIGW], F32R, kind="ExternalInput").ap()
    cur_d = nc.dram_tensor("curn", [BPC, P, N], F32R, kind="ExternalInput").ap()
    wkv_d = nc.dram_tensor("wkv", [128, 256], F32R, kind="ExternalInput").ap()
    wq_d = nc.dram_tensor("wq", [128, 128], F32R, kind="ExternalInput").ap()
    wql_d = nc.dram_tensor("wql", [1, 128], F32R, kind="ExternalInput").ap()
    # consts[128, 3]: col0 = scale_ea (c1), col1 = scale_tanh (c2), col2 = 1/(sqrt(D)*c2_eff)
    cst_d = nc.dram_tensor("cst", [128, 3], F32, kind="ExternalInput").ap()
    if has_mask:
        mask_d = nc.dram_tensor("maskn", [BPC, P, N], F32, kind="ExternalInput").ap()
    out_d = nc.dram_tensor("out", [BPC, P, N], F32, kind="ExternalOutput").ap()

    from contextlib import ExitStack
    with tile.TileContext(nc) as tc, ExitStack() as ctx:
        consts = ctx.enter_context(tc.tile_pool(name="consts", bufs=1))
        io_pool = ctx.enter_context(tc.tile_pool(name="io", bufs=2))
        work = ctx.enter_context(tc.tile_pool(name="work", bufs=2))
        small = ctx.enter_context(tc.tile_pool(name="small", bufs=2))
        psA = ctx.enter_context(tc.tile_pool(name="psA", bufs=3, space="PSUM"))
        psB = ctx.enter_context(tc.tile_pool(name="psB", bufs=2, space="PSUM"))

        ident = consts.tile([128, 128], F32)
        make_identity(nc, ident[:])
        ident_r = consts.tile([128, 128], F32R)
        nc.vector.tensor_copy(ident_r[:], ident[:])
        wkv_t = consts.tile([128, 256], F32R)
        nc.sync.dma_start(wkv_t[:], wkv_d[:])
        wq_t = consts.tile([128, 128], F32R)
        nc.sync.dma_start(wq_t[:], wq_d[:])
        wql_t = consts.tile([1, 128], F32R)
        nc.sync.dma_start(wql_t[:], wql_d[:])
        cst_t = consts.tile([128, 3], F32)
        nc.sync.dma_start(cst_t[:], cst_d[:])
        c1_ap = cst_t[:, 0:1]          # ACT scale for ea
        c2_ap = cst_t[0:PCH, 1:2]      # ACT scale for tanh
        sdc2_ap = cst_t[0:PCH, 2:3]    # fold for sigq'

        rep_ctx = tc.For_i(0, repeat, 1, hint_engines=(
            mybir.EngineType.PE, mybir.EngineType.DVE, mybir.EngineType.Activation,
            mybir.EngineType.SP, mybir.EngineType.Pool)) if repeat > 1 else None
        if rep_ctx is not None:
            ctx.enter_context(rep_ctx)
        for j in range(BPC):
            # ---------- loads (one packed DMA on the SP ring) ----------
            big_t = io_pool.tile([128, BIGW], F32R, tag="bigin")
            nc.sync.dma_start(big_t[:], big_d[j])
            enT_t = big_t[:, 0:N]
            curT_t = big_t[0:CHK, N:N + NCH * P].bitcast(F32)
            egT_t = big_t[:, N + NCH * P:N + NCH * P + P]
            cap_t = big_t[0:1, N + NCH * P + P:N + NCH * P + 2 * P]

            # ---------- eaT = exp(c1 * curT) ----------
            eaT_t = work.tile([CHK, NCH * P], F32R, tag="eaT")
            nc.scalar.activation(eaT_t[:], curT_t[:], AF.Exp, scale=cst_t[0:CHK, 0:1])

            # ---------- k/v -> ekkv ----------
            ekkv_t = work.tile([CHK, NCH * 256], F32R, tag="ekkv")
            for r in range(4):  # 4 rounds x 4 chunks
                kvps = psA.tile([CHK, 1024], F32, tag="big")
                for qq in range(4):
                    c = r * 4 + qq
                    nc.tensor.matmul(kvps[:, qq * 256:(qq + 1) * 256],
                                     enT_t[:, c * CHK:(c + 1) * CHK],
                                     wkv_t[:], start=True, stop=True)
                kv_v = kvps.rearrange("a (q t) -> a q t", t=256)
                out_v = ekkv_t[:, r * 1024:(r + 1) * 1024].rearrange(
                    "a (q t) -> a q t", t=256)
                # ek = exp(k)
                nc.scalar.activation(out_v[:, :, 0:128], kv_v[:, :, 0:128], AF.Exp)
                # ekv = ek * v
                nc.vector.tensor_mul(out_v[:, :, 128:256],
                                     out_v[:, :, 0:128].bitcast(F32),
                                     kv_v[:, :, 128:256])

            # ---------- q -> sigq' ----------
            sig_t = small.tile([PCH, 256], F32, tag="sig")  # [*,0:128]=pc0, [*,128:]=pc1
            for pc in range(2):
                qps = psB.tile([PCH, 128], F32, tag="sm")
                nc.tensor.matmul(qps[:], egT_t[:, pc * PCH:(pc + 1) * PCH],
                                 wq_t[:], start=True, stop=False)
                nc.tensor.matmul(qps[:], cap_t[:, pc * PCH:(pc + 1) * PCH],
                                 wql_t[:], start=False, stop=True)
                eq_t = small.tile([PCH, 128], F32, tag="eq")
                nc.scalar.activation(eq_t[:], qps[:], AF.Exp, scale=-1.0)
                sp_t = small.tile([PCH, 128], F32, tag="sp")
                nc.vector.tensor_scalar(sp_t[:], eq_t[:], 1.0, sdc2_ap,
                                        ALU.add, ALU.mult)
                nc.vector.reciprocal_approx_fast(
                    sig_t[:, pc * 128:(pc + 1) * 128], sp_t[:])

            # ---------- bias/denom -> aftT ----------
            aftT_t = small.tile([128, P], F32R, tag="aftT")
            eaT_v = eaT_t.rearrange("a (c p) -> a c p", p=P)
            for pc in range(2):
                bdps = psB.tile([PCH, 256], F32, tag="sm")
                for c in range(NCH):
                    nc.tensor.matmul(bdps[:], eaT_v[:, c, pc * PCH:(pc + 1) * PCH],
                                     ekkv_t[:, c * 256:(c + 1) * 256],
                                     start=(c == 0), stop=(c == NCH - 1))
                rd_t = small.tile([PCH, 128], F32, tag="rd")
                nc.vector.reciprocal_approx_fast(rd_t[:], bdps[:, 0:128])
                wt_t = small.tile([PCH, 128], F32, tag="wt")
                nc.vector.tensor_mul(wt_t[:], bdps[:, 128:256], rd_t[:])
                aft_t = small.tile([PCH, 128], F32, tag="aft")
                nc.vector.tensor_mul(aft_t[:], wt_t[:],
                                     sig_t[:, pc * 128:(pc + 1) * 128])
                trps = psB.tile([128, PCH], F32, tag="sm")
                nc.tensor.transpose(trps[:], aft_t[:], ident[0:PCH, 0:PCH])
                nc.vector.tensor_copy(aftT_t[:, pc * PCH:(pc + 1) * PCH], trps[:])

            # ---------- score + softmax ----------
            for pc in range(2):
                curn_t = work.tile([PCH, N], F32R, tag="curn", bufs=3)
                nc.gpsimd.dma_start(curn_t[:], cur_d[j, pc * PCH:(pc + 1) * PCH, :])
                if has_mask:
                    mkn_t = work.tile([PCH, N], F32, tag="mkn", bufs=3)
                    nc.gpsimd.dma_start(mkn_t[:], mask_d[j, pc * PCH:(pc + 1) * PCH, :])
                th_t = work.tile([PCH, N], F32, tag="th")
                # bank-aligned score blocks; cur folded into psum via identity matmul
                for b0, bw in ((0, 1024), (1024, 976)):
                    sps = psA.tile([PCH, bw], F32, tag="big")
                    for o0 in range(0, bw, 512):
                        w = min(512, bw - o0)
                        nc.tensor.matmul(sps[:, o0:o0 + w],
                                         aftT_t[:, pc * PCH:(pc + 1) * PCH],
                                         enT_t[:, b0 + o0:b0 + o0 + w],
                                         start=True, stop=False)
                        nc.tensor.matmul(sps[:, o0:o0 + w],
                                         ident_r[0:PCH, 0:PCH],
                                         curn_t[:, b0 + o0:b0 + o0 + w],
                                         start=False, stop=True)
                    nc.scalar.activation(th_t[:, b0:b0 + bw], sps[:], AF.Tanh,
                                         scale=c2_ap)
                e_t = work.tile([PCH, N], F32, tag="et")
                rs_t = small.tile([PCH, 1], F32, tag="rs")
                if has_mask:
                    u_t = work.tile([PCH, N], F32, tag="ut")
                    nc.vector.tensor_scalar_mul(u_t[:], th_t[:], CLIP)
                    nc.vector.tensor_add(u_t[:], u_t[:], mkn_t[:])
                    nc.scalar.activation(e_t[:], u_t[:], AF.Exp, accum_out=rs_t[:])
                else:
                    nc.scalar.activation(e_t[:], th_t[:], AF.Exp, scale=CLIP,
                                         accum_out=rs_t[:])
                rr_t = small.tile([PCH, 1], F32, tag="rr")
                nc.vector.reciprocal(rr_t[:], rs_t[:])
                nc.vector.tensor_scalar_mul(e_t[:], e_t[:], rr_t[:])
                nc.gpsimd.dma_start(out_d[j, pc * PCH:(pc + 1) * PCH, :], e_t[:])

    nc.compile()
    return nc


def get_compiled(has_mask: bool, repeat: int = 1):
    key = ("k", has_mask, repeat)
    if key not in _CACHE:
        _CACHE[key] = _build(has_mask, repeat)
    return _CACHE[key]


def prep_inputs(inputs):
    """Host-side shard + layout prep. Returns (in_maps, has_mask)."""
    eg = np.asarray(inputs["encoded_graph_mean_pomo"], np.float32)   # [B,P,D]
    cap = np.asarray(inputs["capacity"], np.float32)                 # [B,P]
    cur = np.ascontiguousarray(np.asarray(inputs["cur_dist"], np.float32))  # [B,P,N]
    ls = float(np.asarray(inputs["log_scale"]).reshape(-1)[0])
    mask = np.asarray(inputs["ninf_mask"], np.float32)               # [B,P,N]
    en = np.asarray(inputs["encoded_nodes"], np.float32)             # [B,N,D]
    wq = np.asarray(inputs["Wq_last"], np.float32)                   # [D,D+1]
    wk = np.asarray(inputs["Wk"], np.float32)                        # [D,D]
    wv = np.asarray(inputs["Wv"], np.float32)                        # [D,D]
    a1 = float(np.asarray(inputs["AFT_dist_alpha"]).reshape(-1)[0])
    a2 = float(np.asarray(inputs["probs_dist_alpha"]).reshape(-1)[0])

    c1 = ls * a1
    c2 = ls * a2
    has_mask = bool(np.any(mask)) or (c2 == 0.0)

    if has_mask:
        # prescaled general path: A = c1*cur + mask (goes inside exp, transposed),
        # S = c2*cur (added to raw score before tanh), mask re-added after clip.
        curT_src = c1 * cur + mask
        cur_nat = c2 * cur
        sc_ea, sc_th = 1.0, 1.0
        mul2 = SQRT_D          # sigq' = sigmoid(q)/sqrt(D)
    else:
        curT_src = cur
        cur_nat = cur
        sc_ea, sc_th = c1, c2
        mul2 = SQRT_D * c2     # sigq' = sigmoid(q)/(sqrt(D)*c2)

    # [B,P,N] -> per-batch packed transpose [128, NCH*P]:
    # tile[k, c*P + p] = cur[b, p, c*128 + k]
    BIGW = N + NCH * P + P + P
    big = np.zeros((B, 128, BIGW), np.float32)
    big[:, :, 0:N] = en.transpose(0, 2, 1)                               # enT
    big[:, 0:CHK, N:N + NCH * P] = (
        curT_src.reshape(B, P, NCH, CHK).transpose(0, 3, 2, 1)
    ).reshape(B, CHK, NCH * P)                                           # curT packed
    big[:, :, N + NCH * P:N + NCH * P + P] = eg.transpose(0, 2, 1)       # egT
    big[:, 0, N + NCH * P + P:N + NCH * P + 2 * P] = cap                 # cap row
    wkv = np.ascontiguousarray(np.concatenate([wk.T, wv.T], axis=1))  # [D,256]
    wq_m = np.ascontiguousarray(wq[:, :D].T)                 # [D,D]
    wql = np.ascontiguousarray(wq[:, D:D + 1].T)             # [1,D]
    cst = np.zeros((128, 3), np.float32)
    cst[:, 0] = sc_ea
    cst[:, 1] = sc_th
    cst[:, 2] = mul2

    in_maps = []
    for c in range(N_CORES):
        s = slice(c * BPC, (c + 1) * BPC)
        m = {
            "big": big[s],
            "curn": np.ascontiguousarray(cur_nat[s]),
            "wkv": wkv,
            "wq": wq_m,
            "wql": wql,
            "cst": cst,
        }
        if has_mask:
            m["maskn"] = np.ascontiguousarray(mask[s])
        in_maps.append(m)
    return in_maps, has_mask


def kernel(**inputs) -> np.ndarray:
    from concourse.bass_utils import run_bass_kernel_spmd
    in_maps, has_mask = prep_inputs(inputs)
    nc = get_compiled(has_mask)
    res = run_bass_kernel_spmd(nc, in_maps, core_ids=list(range(N_CORES)))
    out = np.empty((B, P, N), np.float32)
    for c in range(N_CORES):
        out[c * BPC:(c + 1) * BPC] = res.results[c]["out"]
    return out


# revision 13
# speedup vs baseline: 1.0423x; 1.0423x over previous
"""Trainium2 Bass kernel for nn_KP_Decoder (AFT-style decoder + softmax).

Shards data-parallel over batch B across 8 NeuronCores (8 batches/core).
Per batch b on-device:
  k|v   = en[b] @ [Wk.T | Wv.T]           (float32r matmuls, N=256)
  ek    = exp(k); ekv = ek*v              (ACT exp -> f32r, DVE mul -> f32r)
  q     = cat(egmp,cap) @ Wq.T            (accumulating matmuls K=128 + K=1)
  sigq' = sigmoid(q) / (sqrt(D)*c2)       (ACT exp + DVE tensor_scalar + recip)
  eaT   = exp(c1 * cur.T)                 (ACT exp, scale folded)
  den|b = eaT.T-chunks @ [ek|ekv]         (16 accumulating f32r matmuls, N=256)
  aft'  = sigq' * bias / denom            (recip_approx_fast + muls)
  aftT  = transpose(aft')                 (PE transpose)
  s'    = aftT.T @ enT                    (f32r matmuls, N=500; 1/(sqrt(D)c2) folded)
  t     = tanh(c2*(s' + cur))             (DVE add + ACT tanh, c2 as scale)
  e     = exp(CLIP*t), rowsum             (ACT exp + accum_out)
  probs = e * (1/rowsum)                  (DVE reciprocal + tensor_scalar)
"""
import sys
if '/opt/trn_rl_repo' not in sys.path:
    sys.path.insert(0, '/opt/trn_rl_repo')

import numpy as np

B, P, N, D = 64, 200, 2000, 128
SQRT_D = 11.313708498984761
CLIP = 10.0
N_CORES = 8
BPC = B // N_CORES            # batches per core
NCH = 16                      # n-chunks
CHK = N // NCH                # 125 rows per contraction chunk
PCH = P // 2                  # 100, two p-chunks
SCORE_BLK = 1000              # score psum block (2 banks)
SCORE_MM = 500                # free dim per score matmul (<=512 fp32)

_CACHE = {}


def _build(has_mask: bool, repeat: int = 1, variant: str = 'full'):
    import concourse.bacc as bacc
    import concourse.mybir as mybir
    import concourse.tile as tile
    from concourse.masks import make_identity

    F32 = mybir.dt.float32
    F32R = mybir.dt.float32r
    AF = mybir.ActivationFunctionType
    ALU = mybir.AluOpType

    nc = bacc.Bacc("TRN2", target_bir_lowering=False, debug=False,
                   num_devices=N_CORES)

    # ---- DRAM I/O (per-core shapes) ----
    # packed per-batch load: [0:N]=enT(f32r), then egT(f32r), cap(row0)
    BIGW = N + P + P
    big_d = nc.dram_tensor("big", [BPC, 128, BIGW], F32R, kind="ExternalInput").ap()
    curh_d = nc.dram_tensor("curh", [BPC, CHK, NCH * P], mybir.dt.bfloat16,
                            kind="ExternalInput").ap()
    cur_d = nc.dram_tensor("curn", [BPC, P, N], mybir.dt.uint16, kind="ExternalInput").ap()
    wkv_d = nc.dram_tensor("wkv", [128, 256], F32R, kind="ExternalInput").ap()
    wq_d = nc.dram_tensor("wq", [128, 128], F32R, kind="ExternalInput").ap()
    wql_d = nc.dram_tensor("wql", [1, 128], F32R, kind="ExternalInput").ap()
    # consts[128, 3]: col0 = scale_ea (c1), col1 = scale_tanh (c2), col2 = 1/(sqrt(D)*c2_eff)
    cst_d = nc.dram_tensor("cst", [128, 5], F32, kind="ExternalInput").ap()
    if has_mask:
        mask_d = nc.dram_tensor("maskn", [BPC, P, N], F32, kind="ExternalInput").ap()
    out_d = nc.dram_tensor("out", [BPC, P, N], F32, kind="ExternalOutput").ap()

    from contextlib import ExitStack
    with tile.TileContext(nc) as tc, ExitStack() as ctx:
        consts = ctx.enter_context(tc.tile_pool(name="consts", bufs=1))
        io_pool = ctx.enter_context(tc.tile_pool(name="io", bufs=2))
        work = ctx.enter_context(tc.tile_pool(name="work", bufs=2))
        small = ctx.enter_context(tc.tile_pool(name="small", bufs=2))
        psA = ctx.enter_context(tc.tile_pool(name="psA", bufs=3, space="PSUM"))
        psB = ctx.enter_context(tc.tile_pool(name="psB", bufs=2, space="PSUM"))

        ident = consts.tile([128, 128], F32)
        make_identity(nc, ident[:])
        ident_r = consts.tile([128, 128], F32R)
        nc.vector.tensor_copy(ident_r[:], ident[:])
        wkv_t = consts.tile([128, 256], F32R)
        nc.sync.dma_start(wkv_t[:], wkv_d[:])
        wq_t = consts.tile([128, 128], F32R)
        nc.sync.dma_start(wq_t[:], wq_d[:])
        wql_t = consts.tile([1, 128], F32R)
        nc.sync.dma_start(wql_t[:], wql_d[:])
        cst_t = consts.tile([128, 5], F32)
        nc.sync.dma_start(cst_t[:], cst_d[:])
        c1_ap = cst_t[:, 0:1]          # ACT scale for ea
        c2_ap = cst_t[0:PCH, 1:2]      # ACT scale for tanh
        sdc2_ap = cst_t[0:PCH, 2:3]    # fold for sigq'

        rep_ctx = tc.For_i(0, repeat, 1, hint_engines=(
            mybir.EngineType.PE, mybir.EngineType.DVE, mybir.EngineType.Activation,
            mybir.EngineType.SP, mybir.EngineType.Pool)) if repeat > 1 else None
        if rep_ctx is not None:
            ctx.enter_context(rep_ctx)
        for j in range(BPC):
            if variant == 'compute_off':
                big_t = io_pool.tile([128, BIGW], F32R, tag="bigin")
                nc.sync.dma_start(big_t[:], big_d[j])
                for pc in range(2):
                    cu16_t = work.tile([PCH, N], mybir.dt.uint16, tag="cu16", bufs=3)
                    nc.scalar.dma_start(cu16_t[:], cur_d[j, pc * PCH:(pc + 1) * PCH, :])
                    e_t = work.tile([PCH, N], F32, tag="et")
                    nc.vector.tensor_copy(e_t[:, 0:16], cu16_t[:, 0:32].bitcast(F32))
                    nc.gpsimd.dma_start(out_d[j, pc * PCH:(pc + 1) * PCH, :], e_t[:])
                continue
            # ---------- loads (one packed DMA on the SP ring) ----------
            big_t = io_pool.tile([128, BIGW], F32R, tag="bigin")
            if variant == 'dma_light':
                nc.sync.dma_start(big_t[:, 0:16], big_d[j][:, 0:16])
            else:
                nc.sync.dma_start(big_t[:], big_d[j])
            enT_t = big_t[:, 0:N]
            egT_t = big_t[:, N:N + P]
            cap_t = big_t[0:1, N + P:N + 2 * P]
            curT_t = io_pool.tile([CHK, NCH * P], mybir.dt.bfloat16, tag="curh")
            if variant == 'dma_light':
                nc.sync.dma_start(curT_t[:, 0:16], curh_d[j][:, 0:16])
            else:
                nc.sync.dma_start(curT_t[:], curh_d[j])

            # ---------- eaT = exp(c1 * curT) ----------
            eaT_t = work.tile([CHK, NCH * P], F32R, tag="eaT")
            nc.scalar.activation(eaT_t[:], curT_t[:], AF.Exp, scale=cst_t[0:CHK, 0:1])

            # ---------- k/v -> ekkv ----------
            ekkv_t = work.tile([CHK, NCH * 256], F32R, tag="ekkv")
            for r in range(4):  # 4 rounds x 4 chunks
                kvps = psA.tile([CHK, 1024], F32, tag="big")
                for qq in range(4):
                    c = r * 4 + qq
                    nc.tensor.matmul(kvps[:, qq * 256:(qq + 1) * 256],
                                     enT_t[:, c * CHK:(c + 1) * CHK],
                                     wkv_t[:], start=True, stop=True)
                kv_v = kvps.rearrange("a (q t) -> a q t", t=256)
                out_v = ekkv_t[:, r * 1024:(r + 1) * 1024].rearrange(
                    "a (q t) -> a q t", t=256)
                # ek = exp(k)
                nc.scalar.activation(out_v[:, :, 0:128], kv_v[:, :, 0:128], AF.Exp)
                # ekv = ek * v
                nc.vector.tensor_mul(out_v[:, :, 128:256],
                                     out_v[:, :, 0:128].bitcast(F32),
                                     kv_v[:, :, 128:256])

            # ---------- q -> sigq' ----------
            sig_t = small.tile([PCH, 256], F32, tag="sig")  # [*,0:128]=pc0, [*,128:]=pc1
            for pc in range(2):
                qps = psB.tile([PCH, 128], F32, tag="sm")
                nc.tensor.matmul(qps[:], egT_t[:, pc * PCH:(pc + 1) * PCH],
                                 wq_t[:], start=True, stop=False)
                nc.tensor.matmul(qps[:], cap_t[:, pc * PCH:(pc + 1) * PCH],
                                 wql_t[:], start=False, stop=True)
                eq_t = small.tile([PCH, 128], F32, tag="eq")
                nc.scalar.activation(eq_t[:], qps[:], AF.Exp, scale=-1.0)
                sp_t = small.tile([PCH, 128], F32, tag="sp")
                nc.vector.tensor_scalar(sp_t[:], eq_t[:], 1.0, sdc2_ap,
                                        ALU.add, ALU.mult)
                nc.vector.reciprocal_approx_fast(
                    sig_t[:, pc * 128:(pc + 1) * 128], sp_t[:])

            # ---------- bias/denom -> aftT ----------
            aftT_t = small.tile([128, P], F32R, tag="aftT")
            eaT_v = eaT_t.rearrange("a (c p) -> a c p", p=P)
            for pc in range(2):
                bdps = psB.tile([PCH, 256], F32, tag="sm")
                for c in range(NCH):
                    nc.tensor.matmul(bdps[:], eaT_v[:, c, pc * PCH:(pc + 1) * PCH],
                                     ekkv_t[:, c * 256:(c + 1) * 256],
                                     start=(c == 0), stop=(c == NCH - 1))
                rd_t = small.tile([PCH, 128], F32, tag="rd")
                nc.vector.reciprocal_approx_fast(rd_t[:], bdps[:, 0:128])
                wt_t = small.tile([PCH, 128], F32, tag="wt")
                nc.vector.tensor_mul(wt_t[:], bdps[:, 128:256], rd_t[:])
                aft_t = small.tile([PCH, 128], F32, tag="aft")
                nc.vector.tensor_mul(aft_t[:], wt_t[:],
                                     sig_t[:, pc * 128:(pc + 1) * 128])
                trps = psB.tile([128, PCH], F32, tag="sm")
                nc.tensor.transpose(trps[:], aft_t[:], ident[0:PCH, 0:PCH])
                nc.vector.tensor_copy(aftT_t[:, pc * PCH:(pc + 1) * PCH], trps[:])

            # ---------- score + softmax ----------
            for pc in range(2):
                cu16_t = work.tile([PCH, N], mybir.dt.uint16, tag="cu16", bufs=3)
                if variant == 'dma_light':
                    nc.scalar.dma_start(cu16_t[:, 0:16],
                                        cur_d[j, pc * PCH:(pc + 1) * PCH, 0:16])
                else:
                    nc.scalar.dma_start(cu16_t[:], cur_d[j, pc * PCH:(pc + 1) * PCH, :])
                curn_t = work.tile([PCH, N], F32R, tag="curn", bufs=2)
                nc.vector.tensor_scalar(curn_t[:], cu16_t[:], cst_t[0:PCH, 3:4],
                                        cst_t[0:PCH, 4:5], ALU.mult, ALU.add)
                if has_mask:
                    mkn_t = work.tile([PCH, N], F32, tag="mkn", bufs=3)
                    nc.gpsimd.dma_start(mkn_t[:], mask_d[j, pc * PCH:(pc + 1) * PCH, :])
                th_t = work.tile([PCH, N], F32, tag="th")
                # bank-aligned score blocks; cur folded into psum via identity matmul
                for b0, bw in ((0, 1024), (1024, 976)):
                    sps = psA.tile([PCH, bw], F32, tag="big")
                    for o0 in range(0, bw, 512):
                        w = min(512, bw - o0)
                        nc.tensor.matmul(sps[:, o0:o0 + w],
                                         aftT_t[:, pc * PCH:(pc + 1) * PCH],
                                         enT_t[:, b0 + o0:b0 + o0 + w],
                                         start=True, stop=False)
                        nc.tensor.matmul(sps[:, o0:o0 + w],
                                         ident_r[0:PCH, 0:PCH],
                                         curn_t[:, b0 + o0:b0 + o0 + w],
                                         start=False, stop=True)
                    nc.scalar.activation(th_t[:, b0:b0 + bw], sps[:], AF.Tanh,
                                         scale=c2_ap)
                e_t = work.tile([PCH, N], F32, tag="et")
                rs_t = small.tile([PCH, 1], F32, tag="rs")
                if has_mask:
                    u_t = work.tile([PCH, N], F32, tag="ut")
                    nc.vector.tensor_scalar_mul(u_t[:], th_t[:], CLIP)
                    nc.vector.tensor_add(u_t[:], u_t[:], mkn_t[:])
                    nc.scalar.activation(e_t[:], u_t[:], AF.Exp, accum_out=rs_t[:])
                else:
                    nc.scalar.activation(e_t[:], th_t[:], AF.Exp, scale=CLIP,
                                         accum_out=rs_t[:])
                rr_t = small.tile([PCH, 1], F32, tag="rr")
                nc.vector.reciprocal(rr_t[:], rs_t[:])
                nc.vector.tensor_scalar_mul(e_t[:], e_t[:], rr_t[:])
                if variant == 'dma_light':
                    nc.gpsimd.dma_start(out_d[j, pc * PCH:(pc + 1) * PCH, 0:16],
                                        e_t[:, 0:16])
                else:
                    nc.gpsimd.dma_start(out_d[j, pc * PCH:(pc + 1) * PCH, :], e_t[:])

    nc.compile()
    return nc


def get_compiled(has_mask: bool, repeat: int = 1, variant: str = 'full'):
    key = ("k", has_mask, repeat, variant)
    if key not in _CACHE:
        _CACHE[key] = _build(has_mask, repeat, variant)
    return _CACHE[key]


def prep_inputs(inputs):
    """Host-side shard + layout prep. Returns (in_maps, has_mask)."""
    eg = np.asarray(inputs["encoded_graph_mean_pomo"], np.float32)   # [B,P,D]
    cap = np.asarray(inputs["capacity"], np.float32)                 # [B,P]
    cur = np.ascontiguousarray(np.asarray(inputs["cur_dist"], np.float32))  # [B,P,N]
    ls = float(np.asarray(inputs["log_scale"]).reshape(-1)[0])
    mask = np.asarray(inputs["ninf_mask"], np.float32)               # [B,P,N]
    en = np.asarray(inputs["encoded_nodes"], np.float32)             # [B,N,D]
    wq = np.asarray(inputs["Wq_last"], np.float32)                   # [D,D+1]
    wk = np.asarray(inputs["Wk"], np.float32)                        # [D,D]
    wv = np.asarray(inputs["Wv"], np.float32)                        # [D,D]
    a1 = float(np.asarray(inputs["AFT_dist_alpha"]).reshape(-1)[0])
    a2 = float(np.asarray(inputs["probs_dist_alpha"]).reshape(-1)[0])

    c1 = ls * a1
    c2 = ls * a2
    has_mask = bool(np.any(mask)) or (c2 == 0.0)

    if has_mask:
        # prescaled general path: A = c1*cur + mask (goes inside exp, transposed),
        # S = c2*cur (added to raw score before tanh), mask re-added after clip.
        curT_src = c1 * cur + mask
        cur_nat = c2 * cur
        sc_ea, sc_th = 1.0, 1.0
        mul2 = SQRT_D          # sigq' = sigmoid(q)/sqrt(D)
    else:
        curT_src = cur
        cur_nat = cur
        sc_ea, sc_th = c1, c2
        mul2 = SQRT_D * c2     # sigq' = sigmoid(q)/(sqrt(D)*c2)

    import ml_dtypes
    # [B,P,N] -> per-batch packed transpose; curT shipped as bf16
    BIGW = N + P + P
    big = np.zeros((B, 128, BIGW), np.float32)
    big[:, :, 0:N] = en.transpose(0, 2, 1)                               # enT
    curh = np.ascontiguousarray(
        curT_src.reshape(B, P, NCH, CHK).transpose(0, 3, 2, 1)
    ).reshape(B, CHK, NCH * P).astype(ml_dtypes.bfloat16)
    big[:, :, N:N + P] = eg.transpose(0, 2, 1)                           # egT
    big[:, 0, N + P:N + 2 * P] = cap                                     # cap row

    # curn: uint16 fixed point of cur_nat over [lo, hi]
    lo = float(cur_nat.min())
    hi = float(cur_nat.max())
    if not np.isfinite(lo) or not np.isfinite(hi) or hi <= lo:
        lo, hi = lo if np.isfinite(lo) else 0.0, (lo if np.isfinite(lo) else 0.0) + 1.0
    cq = ((cur_nat - lo) * (65535.0 / (hi - lo))).round().astype(np.uint16)

    wkv = np.ascontiguousarray(np.concatenate([wk.T, wv.T], axis=1))  # [D,256]
    wq_m = np.ascontiguousarray(wq[:, :D].T)                 # [D,D]
    wql = np.ascontiguousarray(wq[:, D:D + 1].T)             # [1,D]
    cst = np.zeros((128, 5), np.float32)
    cst[:, 0] = sc_ea
    cst[:, 1] = sc_th
    cst[:, 2] = mul2
    cst[:, 3] = (hi - lo) / 65535.0
    cst[:, 4] = lo

    in_maps = []
    for c in range(N_CORES):
        s = slice(c * BPC, (c + 1) * BPC)
        m = {
            "big": big[s],
            "curh": curh[s],
            "curn": cq[s],
            "wkv": wkv,
            "wq": wq_m,
            "wql": wql,
            "cst": cst,
        }
        if has_mask:
            m["maskn"] = np.ascontiguousarray(mask[s])
        in_maps.append(m)
    return in_maps, has_mask


def kernel(**inputs) -> np.ndarray:
    from concourse.bass_utils import run_bass_kernel_spmd
    in_maps, has_mask = prep_inputs(inputs)
    nc = get_compiled(has_mask)
    res = run_bass_kernel_spmd(nc, in_maps, core_ids=list(range(N_CORES)))
    out = np.empty((B, P, N), np.float32)
    for c in range(N_CORES):
        out[c * BPC:(c + 1) * BPC] = res.results[c]["out"]
    return out



# revision 18
# speedup vs baseline: 1.1118x; 1.0667x over previous
"""Trainium2 Bass kernel for nn_KP_Decoder (AFT-style decoder + softmax).

Shards data-parallel over batch B across 8 NeuronCores (8 batches/core).
Per batch b on-device:
  k|v   = en[b] @ [Wk.T | Wv.T]           (float32r matmuls, N=256)
  ek    = exp(k); ekv = ek*v              (ACT exp -> f32r, DVE mul -> f32r)
  q     = cat(egmp,cap) @ Wq.T            (accumulating matmuls K=128 + K=1)
  sigq' = sigmoid(q) / (sqrt(D)*c2)       (ACT exp + DVE tensor_scalar + recip)
  eaT   = exp(c1 * cur.T)                 (ACT exp on bf16 curT, scale folded)
  den|b = eaT.T-chunks @ [ek|ekv]         (16 accumulating f32r matmuls, N=256)
  aft'  = sigq' * bias / denom            (recip_approx_fast + muls)
  aftT  = transpose(aft')                 (PE transpose)
  s'    = aftT.T @ enT + cur              (f32r matmuls; cur added via identity matmul)
  t     = tanh(c2*s')                     (ACT tanh reads PSUM, c2 as scale)
  e     = exp(CLIP*t), rowsum             (ACT exp + accum_out)
  probs = e * (1/rowsum)                  (DVE reciprocal + tensor_scalar)

cur_dist is shipped twice in compressed form: bf16 transposed (exp path --
error cancels in the bias/denom ratio) and uint16 fixed-point natural
(score path, dequantized on-chip to f32r).
"""
import sys
if '/opt/trn_rl_repo' not in sys.path:
    sys.path.insert(0, '/opt/trn_rl_repo')

import numpy as np

B, P, N, D = 64, 200, 2000, 128
SQRT_D = 11.313708498984761
CLIP = 10.0
N_CORES = 8
BPC = B // N_CORES            # batches per core
NCH = 16                      # n-chunks
CHK = N // NCH                # 125 rows per contraction chunk
PCH = P // 2                  # 100, two p-chunks

_CACHE = {}


def _build(has_mask: bool, repeat: int = 1, variant: str = 'full'):
    import concourse.bacc as bacc
    import concourse.mybir as mybir
    import concourse.tile as tile
    from concourse.masks import make_identity

    F32 = mybir.dt.float32
    F32R = mybir.dt.float32r
    BF16 = mybir.dt.bfloat16
    U16 = mybir.dt.uint16
    AF = mybir.ActivationFunctionType
    ALU = mybir.AluOpType

    DMA_ON = 'dma_light' not in variant
    ACT_ON = 'act_off' not in variant
    DVE_ON = 'dve_off' not in variant
    PE_ON = 'pe_off' not in variant

    nc = bacc.Bacc("TRN2", target_bir_lowering=False, debug=False,
                   num_devices=N_CORES)

    # ---- DRAM I/O (per-core shapes) ----
    BIGW = N + P + P  # packed: [0:N]=enT(f32r), egT(f32r), cap(row0)
    big_d = nc.dram_tensor("big", [BPC, 128, BIGW], F32R, kind="ExternalInput").ap()
    curh_d = nc.dram_tensor("curh", [BPC, CHK, NCH * P], BF16, kind="ExternalInput").ap()
    cur_d = nc.dram_tensor("curn", [BPC, P, N], U16, kind="ExternalInput").ap()
    wkv_d = nc.dram_tensor("wkv", [128, 256], F32R, kind="ExternalInput").ap()
    wq_d = nc.dram_tensor("wq", [128, 128], F32R, kind="ExternalInput").ap()
    wql_d = nc.dram_tensor("wql", [1, 128], F32R, kind="ExternalInput").ap()
    # consts[128, 5]: scale_ea(c1), scale_tanh(c2), sqrt(D)*c2_eff, cur_scale, cur_lo
    cst_d = nc.dram_tensor("cst", [128, 5], F32, kind="ExternalInput").ap()
    if has_mask:
        mask_d = nc.dram_tensor("maskn", [BPC, P, N], F32, kind="ExternalInput").ap()
    out_d = nc.dram_tensor("out", [BPC, P, N], F32, kind="ExternalOutput").ap()

    from contextlib import ExitStack
    with tile.TileContext(nc) as tc, ExitStack() as ctx:
        consts = ctx.enter_context(tc.tile_pool(name="consts", bufs=1))
        io_pool = ctx.enter_context(tc.tile_pool(name="io", bufs=3))
        work = ctx.enter_context(tc.tile_pool(name="work", bufs=2))
        small = ctx.enter_context(tc.tile_pool(name="small", bufs=2))
        psA = ctx.enter_context(tc.tile_pool(name="psA", bufs=3, space="PSUM"))
        psB = ctx.enter_context(tc.tile_pool(name="psB", bufs=2, space="PSUM"))

        ident = consts.tile([128, 128], F32)
        make_identity(nc, ident[:])
        ident_r = consts.tile([128, 128], F32R)
        nc.vector.tensor_copy(ident_r[:], ident[:])
        wkv_t = consts.tile([128, 256], F32R)
        nc.sync.dma_start(wkv_t[:], wkv_d[:])
        wq_t = consts.tile([128, 128], F32R)
        nc.sync.dma_start(wq_t[:], wq_d[:])
        wql_t = consts.tile([1, 128], F32R)
        nc.sync.dma_start(wql_t[:], wql_d[:])
        cst_t = consts.tile([128, 5], F32)
        nc.sync.dma_start(cst_t[:], cst_d[:])
        c2_ap = cst_t[0:PCH, 1:2]      # ACT scale for tanh
        sdc2_ap = cst_t[0:PCH, 2:3]    # fold for sigq'

        rep_ctx = tc.For_i(0, repeat, 1, hint_engines=(
            mybir.EngineType.PE, mybir.EngineType.DVE, mybir.EngineType.Activation,
            mybir.EngineType.SP, mybir.EngineType.Pool)) if repeat > 1 else None
        if rep_ctx is not None:
            ctx.enter_context(rep_ctx)
        for j in range(BPC):
            # ---------- loads ----------
            big_t = io_pool.tile([128, BIGW], F32R, tag="bigin")
            curT_t = io_pool.tile([CHK, NCH * P], BF16, tag="curh")
            if DMA_ON:
                nc.sync.dma_start(big_t[:], big_d[j])
                nc.sync.dma_start(curT_t[:], curh_d[j])
            else:
                nc.sync.dma_start(big_t[:, 0:16], big_d[j][:, 0:16])
                nc.sync.dma_start(curT_t[:, 0:16], curh_d[j][:, 0:16])
            enT_t = big_t[:, 0:N]
            egT_t = big_t[:, N:N + P]
            cap_t = big_t[0:1, N + P:N + 2 * P]

            # ---------- eaT = exp(c1 * curT) ----------
            eaT_t = work.tile([CHK, NCH * P], F32R, tag="eaT")
            if ACT_ON:
                nc.scalar.activation(eaT_t[:], curT_t[:], AF.Exp,
                                     scale=cst_t[0:CHK, 0:1])

            # ---------- k/v -> ekkv ----------
            ekkv_t = work.tile([CHK, NCH * 256], F32R, tag="ekkv")
            for r in range(4):  # 4 rounds x 4 chunks
                kvps = psA.tile([CHK, 1024], F32, tag="big")
                if PE_ON:
                    for qq in range(4):
                        c = r * 4 + qq
                        nc.tensor.matmul(kvps[:, qq * 256:(qq + 1) * 256],
                                         enT_t[:, c * CHK:(c + 1) * CHK],
                                         wkv_t[:], start=True, stop=True)
                kv_v = kvps.rearrange("a (q t) -> a q t", t=256)
                out_v = ekkv_t[:, r * 1024:(r + 1) * 1024].rearrange(
                    "a (q t) -> a q t", t=256)
                if ACT_ON:
                    nc.scalar.activation(out_v[:, :, 0:128], kv_v[:, :, 0:128], AF.Exp)
                if DVE_ON:
                    nc.vector.tensor_mul(out_v[:, :, 128:256],
                                         out_v[:, :, 0:128].bitcast(F32),
                                         kv_v[:, :, 128:256])

            # ---------- q -> sigq' ----------
            sig_t = small.tile([PCH, 256], F32, tag="sig")
            qps = psB.tile([PCH, 256], F32, tag="sm")
            if PE_ON:
                for pc in range(2):
                    qsl = qps[:, pc * 128:(pc + 1) * 128]
                    nc.tensor.matmul(qsl, egT_t[:, pc * PCH:(pc + 1) * PCH],
                                     wq_t[:], start=True, stop=False)
                    nc.tensor.matmul(qsl, cap_t[:, pc * PCH:(pc + 1) * PCH],
                                     wql_t[:], start=False, stop=True)
            eq_t = small.tile([PCH, 256], F32, tag="eq")
            if ACT_ON:
                nc.scalar.activation(eq_t[:], qps[:], AF.Exp, scale=-1.0)
            if DVE_ON:
                sp_t = small.tile([PCH, 256], F32, tag="sp")
                nc.vector.tensor_scalar(sp_t[:], eq_t[:], 1.0, sdc2_ap,
                                        ALU.add, ALU.mult)
                nc.vector.reciprocal_approx_fast(sig_t[:], sp_t[:])

            # ---------- bias/denom -> aftT ----------
            aftT_t = small.tile([128, P], F32R, tag="aftT")
            eaT_v = eaT_t.rearrange("a (c p) -> a c p", p=P)
            for pc in range(2):
                bdps = psB.tile([PCH, 256], F32, tag="sm")
                if PE_ON:
                    for c in range(NCH):
                        nc.tensor.matmul(bdps[:],
                                         eaT_v[:, c, pc * PCH:(pc + 1) * PCH],
                                         ekkv_t[:, c * 256:(c + 1) * 256],
                                         start=(c == 0), stop=(c == NCH - 1))
                aft_t = small.tile([PCH, 128], F32, tag="aft")
                if DVE_ON:
                    rd_t = small.tile([PCH, 128], F32, tag="rd")
                    nc.vector.reciprocal_approx_fast(rd_t[:], bdps[:, 0:128])
                    wt_t = small.tile([PCH, 128], F32, tag="wt")
                    nc.vector.tensor_mul(wt_t[:], bdps[:, 128:256], rd_t[:])
                    nc.vector.tensor_mul(aft_t[:], wt_t[:],
                                         sig_t[:, pc * 128:(pc + 1) * 128])
                trps = psB.tile([128, PCH], F32, tag="sm")
                if PE_ON:
                    nc.tensor.transpose(trps[:], aft_t[:], ident[0:PCH, 0:PCH])
                if DVE_ON:
                    nc.vector.tensor_copy(aftT_t[:, pc * PCH:(pc + 1) * PCH], trps[:])

            # ---------- score + softmax ----------
            for pc in range(2):
                cu16_t = work.tile([PCH, N], U16, tag="cu16", bufs=3)
                if DMA_ON:
                    nc.scalar.dma_start(cu16_t[:], cur_d[j, pc * PCH:(pc + 1) * PCH, :])
                else:
                    nc.scalar.dma_start(cu16_t[:, 0:16],
                                        cur_d[j, pc * PCH:(pc + 1) * PCH, 0:16])
                curn_t = work.tile([PCH, N], F32R, tag="curn", bufs=2)
                if DVE_ON:
                    nc.gpsimd.tensor_scalar(curn_t[:], cu16_t[:], cst_t[0:PCH, 3:4],
                                            cst_t[0:PCH, 4:5], ALU.mult, ALU.add)
                if has_mask:
                    mkn_t = work.tile([PCH, N], F32, tag="mkn", bufs=3)
                    if DMA_ON:
                        nc.gpsimd.dma_start(mkn_t[:],
                                            mask_d[j, pc * PCH:(pc + 1) * PCH, :])
                    else:
                        nc.gpsimd.dma_start(mkn_t[:, 0:16],
                                            mask_d[j, pc * PCH:(pc + 1) * PCH, 0:16])
                th_t = work.tile([PCH, N], F32, tag="th")
                # bank-aligned score blocks; cur folded into psum via identity matmul
                for b0, bw in ((0, 1024), (1024, 976)):
                    sps = psA.tile([PCH, bw], F32, tag="big")
                    if PE_ON:
                        for o0 in range(0, bw, 512):
                            w = min(512, bw - o0)
                            nc.tensor.matmul(sps[:, o0:o0 + w],
                                             aftT_t[:, pc * PCH:(pc + 1) * PCH],
                                             enT_t[:, b0 + o0:b0 + o0 + w],
                                             start=True, stop=False)
                            nc.tensor.matmul(sps[:, o0:o0 + w],
                                             ident_r[0:PCH, 0:PCH],
                                             curn_t[:, b0 + o0:b0 + o0 + w],
                                             start=False, stop=True)
                    if ACT_ON:
                        nc.scalar.activation(th_t[:, b0:b0 + bw], sps[:], AF.Tanh,
                                             scale=c2_ap)
                e_t = work.tile([PCH, N], F32, tag="et")
                rs_t = small.tile([PCH, 1], F32, tag="rs")
                if has_mask:
                    u_t = work.tile([PCH, N], F32, tag="ut")
                    if DVE_ON:
                        nc.vector.tensor_scalar_mul(u_t[:], th_t[:], CLIP)
                        nc.vector.tensor_add(u_t[:], u_t[:], mkn_t[:])
                    if ACT_ON:
                        nc.scalar.activation(e_t[:], u_t[:], AF.Exp, accum_out=rs_t[:])
                else:
                    if ACT_ON:
                        nc.scalar.activation(e_t[:], th_t[:], AF.Exp, scale=CLIP,
                                             accum_out=rs_t[:])
                if DVE_ON:
                    rr_t = small.tile([PCH, 1], F32, tag="rr")
                    nc.vector.reciprocal(rr_t[:], rs_t[:])
                    nc.vector.tensor_scalar_mul(e_t[:], e_t[:], rr_t[:])
                if DMA_ON:
                    nc.gpsimd.dma_start(out_d[j, pc * PCH:(pc + 1) * PCH, :], e_t[:])
                else:
                    nc.gpsimd.dma_start(out_d[j, pc * PCH:(pc + 1) * PCH, 0:16],
                                        e_t[:, 0:16])

    nc.compile()
    return nc


def get_compiled(has_mask: bool, repeat: int = 1, variant: str = 'full'):
    key = ("k", has_mask, repeat, variant)
    if key not in _CACHE:
        _CACHE[key] = _build(has_mask, repeat, variant)
    return _CACHE[key]


def prep_inputs(inputs):
    """Host-side shard + layout prep. Returns (in_maps, has_mask)."""
    eg = np.asarray(inputs["encoded_graph_mean_pomo"], np.float32)   # [B,P,D]
    cap = np.asarray(inputs["capacity"], np.float32)                 # [B,P]
    cur = np.ascontiguousarray(np.asarray(inputs["cur_dist"], np.float32))  # [B,P,N]
    ls = float(np.asarray(inputs["log_scale"]).reshape(-1)[0])
    mask = np.asarray(inputs["ninf_mask"], np.float32)               # [B,P,N]
    en = np.asarray(inputs["encoded_nodes"], np.float32)             # [B,N,D]
    wq = np.asarray(inputs["Wq_last"], np.float32)                   # [D,D+1]
    wk = np.asarray(inputs["Wk"], np.float32)                        # [D,D]
    wv = np.asarray(inputs["Wv"], np.float32)                        # [D,D]
    a1 = float(np.asarray(inputs["AFT_dist_alpha"]).reshape(-1)[0])
    a2 = float(np.asarray(inputs["probs_dist_alpha"]).reshape(-1)[0])

    c1 = ls * a1
    c2 = ls * a2
    has_mask = bool(np.any(mask)) or (c2 == 0.0)

    if has_mask:
        # prescaled general path: A = c1*cur + mask (goes inside exp, transposed),
        # S = c2*cur (added to raw score before tanh), mask re-added after clip.
        curT_src = c1 * cur + mask
        cur_nat = c2 * cur
        sc_ea, sc_th = 1.0, 1.0
        mul2 = SQRT_D          # sigq' = sigmoid(q)/sqrt(D)
    else:
        curT_src = cur
        cur_nat = cur
        sc_ea, sc_th = c1, c2
        mul2 = SQRT_D * c2     # sigq' = sigmoid(q)/(sqrt(D)*c2)

    import ml_dtypes
    BIGW = N + P + P
    big = np.zeros((B, 128, BIGW), np.float32)
    big[:, :, 0:N] = en.transpose(0, 2, 1)                               # enT
    big[:, :, N:N + P] = eg.transpose(0, 2, 1)                           # egT
    big[:, 0, N + P:N + 2 * P] = cap                                     # cap row
    # curT: per-batch packed transpose, bf16: tile[k, c*P+p] = cur[b, p, c*CHK+k]
    curh = np.ascontiguousarray(
        curT_src.reshape(B, P, NCH, CHK).transpose(0, 3, 2, 1)
    ).reshape(B, CHK, NCH * P).astype(ml_dtypes.bfloat16)

    # curn: uint16 fixed point of cur_nat over [lo, hi]
    lo = float(cur_nat.min())
    hi = float(cur_nat.max())
    if not np.isfinite(lo) or not np.isfinite(hi) or hi <= lo:
        lo = lo if np.isfinite(lo) else 0.0
        hi = lo + 1.0
    cq = ((cur_nat - lo) * (65535.0 / (hi - lo))).round().astype(np.uint16)

    wkv = np.ascontiguousarray(np.concatenate([wk.T, wv.T], axis=1))  # [D,256]
    wq_m = np.ascontiguousarray(wq[:, :D].T)                 # [D,D]
    wql = np.ascontiguousarray(wq[:, D:D + 1].T)             # [1,D]
    cst = np.zeros((128, 5), np.float32)
    cst[:, 0] = sc_ea
    cst[:, 1] = sc_th
    cst[:, 2] = mul2
    cst[:, 3] = (hi - lo) / 65535.0
    cst[:, 4] = lo

    in_maps = []
    for c in range(N_CORES):
        s = slice(c * BPC, (c + 1) * BPC)
        m = {
            "big": big[s],
            "curh": curh[s],
            "curn": cq[s],
            "wkv": wkv,
            "wq": wq_m,
            "wql": wql,
            "cst": cst,
        }
        if has_mask:
            m["maskn"] = np.ascontiguousarray(mask[s])
        in_maps.append(m)
    return in_maps, has_mask


def kernel(**inputs) -> np.ndarray:
    from concourse.bass_utils import run_bass_kernel_spmd
    in_maps, has_mask = prep_inputs(inputs)
    nc = get_compiled(has_mask)
    res = run_bass_kernel_spmd(nc, in_maps, core_ids=list(range(N_CORES)))
    out = np.empty((B, P, N), np.float32)
    for c in range(N_CORES):
        out[c * BPC:(c + 1) * BPC] = res.results[c]["out"]
    return out


# revision 20
# speedup vs baseline: 1.1295x; 1.0159x over previous
"""Trainium2 Bass kernel for nn_KP_Decoder (AFT-style decoder + softmax).

Shards data-parallel over batch B across 8 NeuronCores (8 batches/core).
Per batch b on-device:
  k|v   = en[b] @ [Wk.T | Wv.T]           (float32r matmuls, N=256)
  ek    = exp(k); ekv = ek*v              (ACT exp -> f32r, DVE mul -> f32r)
  q     = cat(egmp,cap) @ Wq.T            (accumulating matmuls K=128 + K=1)
  sigq' = sigmoid(q) / (sqrt(D)*c2)       (ACT exp + DVE tensor_scalar + recip)
  eaT   = exp(c1 * cur.T)                 (ACT exp on bf16 curT, scale folded)
  den|b = eaT.T-chunks @ [ek|ekv]         (16 accumulating f32r matmuls, N=256)
  aft'  = sigq' * bias / denom            (recip_approx_fast + muls)
  aftT  = transpose(aft')                 (PE transpose)
  s'    = aftT.T @ enT + cur              (f32r matmuls; cur added via identity matmul)
  t     = tanh(c2*s')                     (ACT tanh reads PSUM, c2 as scale)
  e     = exp(CLIP*t), rowsum             (ACT exp + accum_out)
  probs = e * (1/rowsum)                  (DVE reciprocal + tensor_scalar)

cur_dist is shipped twice in compressed form: bf16 transposed (exp path --
error cancels in the bias/denom ratio) and uint16 fixed-point natural
(score path, dequantized on-chip to f32r).
"""
import sys
if '/opt/trn_rl_repo' not in sys.path:
    sys.path.insert(0, '/opt/trn_rl_repo')

import numpy as np

B, P, N, D = 64, 200, 2000, 128
SQRT_D = 11.313708498984761
CLIP = 10.0
N_CORES = 8
BPC = B // N_CORES            # batches per core
NCH = 16                      # n-chunks
CHK = N // NCH                # 125 rows per contraction chunk
PCH = P // 2                  # 100, two p-chunks

_CACHE = {}


def _build(has_mask: bool, repeat: int = 1, variant: str = 'full'):
    import concourse.bacc as bacc
    import concourse.mybir as mybir
    import concourse.tile as tile
    from concourse.masks import make_identity

    F32 = mybir.dt.float32
    F32R = mybir.dt.float32r
    BF16 = mybir.dt.bfloat16
    U16 = mybir.dt.uint16
    AF = mybir.ActivationFunctionType
    ALU = mybir.AluOpType

    DMA_ON = 'dma_light' not in variant
    ACT_ON = 'act_off' not in variant
    DVE_ON = 'dve_off' not in variant
    PE_ON = 'pe_off' not in variant

    nc = bacc.Bacc("TRN2", target_bir_lowering=False, debug=False,
                   num_devices=N_CORES)

    # ---- DRAM I/O (per-core shapes) ----
    BIGW = N + P + P  # packed: [0:N]=enT(f32r), egT(f32r), cap(row0)
    big_d = nc.dram_tensor("big", [BPC, 128, BIGW], F32R, kind="ExternalInput").ap()
    # merged 2-byte payload: rows<CHK cols[0:3200]=curT bf16; cols[3200:7200]=cur u16 (2 pchunks)
    C2W = NCH * P + 2 * N
    cu2_d = nc.dram_tensor("cu2", [BPC, 128, C2W], U16, kind="ExternalInput").ap()
    wkv_d = nc.dram_tensor("wkv", [128, 256], F32R, kind="ExternalInput").ap()
    wq_d = nc.dram_tensor("wq", [128, 128], F32R, kind="ExternalInput").ap()
    wql_d = nc.dram_tensor("wql", [1, 128], F32R, kind="ExternalInput").ap()
    # consts[128, 5]: scale_ea(c1), scale_tanh(c2), sqrt(D)*c2_eff, cur_scale, cur_lo
    cst_d = nc.dram_tensor("cst", [128, 5], F32, kind="ExternalInput").ap()
    if has_mask:
        mask_d = nc.dram_tensor("maskn", [BPC, P, N], F32, kind="ExternalInput").ap()
    out_d = nc.dram_tensor("out", [BPC, P, N], F32, kind="ExternalOutput").ap()

    from contextlib import ExitStack
    with tile.TileContext(nc) as tc, ExitStack() as ctx:
        consts = ctx.enter_context(tc.tile_pool(name="consts", bufs=1))
        io_pool = ctx.enter_context(tc.tile_pool(name="io", bufs=3))
        work = ctx.enter_context(tc.tile_pool(name="work", bufs=2))
        small = ctx.enter_context(tc.tile_pool(name="small", bufs=2))
        psA = ctx.enter_context(tc.tile_pool(name="psA", bufs=3, space="PSUM"))
        psB = ctx.enter_context(tc.tile_pool(name="psB", bufs=2, space="PSUM"))

        ident = consts.tile([128, 128], F32)
        make_identity(nc, ident[:])
        ident_r = consts.tile([128, 128], F32R)
        nc.vector.tensor_copy(ident_r[:], ident[:])
        wkv_t = consts.tile([128, 256], F32R)
        nc.sync.dma_start(wkv_t[:], wkv_d[:])
        wq_t = consts.tile([128, 128], F32R)
        nc.sync.dma_start(wq_t[:], wq_d[:])
        wql_t = consts.tile([1, 128], F32R)
        nc.sync.dma_start(wql_t[:], wql_d[:])
        cst_t = consts.tile([128, 5], F32)
        nc.sync.dma_start(cst_t[:], cst_d[:])
        c2_ap = cst_t[0:PCH, 1:2]      # ACT scale for tanh
        sdc2_ap = cst_t[0:PCH, 2:3]    # fold for sigq'

        rep_ctx = tc.For_i(0, repeat, 1, hint_engines=(
            mybir.EngineType.PE, mybir.EngineType.DVE, mybir.EngineType.Activation,
            mybir.EngineType.SP, mybir.EngineType.Pool)) if repeat > 1 else None
        if rep_ctx is not None:
            ctx.enter_context(rep_ctx)
        for j in range(BPC):
            # ---------- loads ----------
            big_t = io_pool.tile([128, BIGW], F32R, tag="bigin")
            c2b_t = io_pool.tile([128, C2W], U16, tag="c2b", bufs=2)
            if DMA_ON:
                nc.sync.dma_start(big_t[:], big_d[j])
                nc.sync.dma_start(c2b_t[:], cu2_d[j])
            else:
                nc.sync.dma_start(big_t[:, 0:16], big_d[j][:, 0:16])
                nc.sync.dma_start(c2b_t[:, 0:16], cu2_d[j][:, 0:16])
            enT_t = big_t[:, 0:N]
            egT_t = big_t[:, N:N + P]
            cap_t = big_t[0:1, N + P:N + 2 * P]
            curT_t = c2b_t[0:CHK, 0:NCH * P].bitcast(BF16)

            # ---------- eaT = exp(c1 * curT) ----------
            eaT_t = work.tile([CHK, NCH * P], F32R, tag="eaT")
            if ACT_ON:
                nc.scalar.activation(eaT_t[:], curT_t[:], AF.Exp,
                                     scale=cst_t[0:CHK, 0:1])

            # ---------- k/v -> ekkv ----------
            ekkv_t = work.tile([CHK, NCH * 256], F32R, tag="ekkv")
            for r in range(4):  # 4 rounds x 4 chunks
                kvps = psA.tile([CHK, 1024], F32, tag="big")
                if PE_ON:
                    for qq in range(4):
                        c = r * 4 + qq
                        nc.tensor.matmul(kvps[:, qq * 256:(qq + 1) * 256],
                                         enT_t[:, c * CHK:(c + 1) * CHK],
                                         wkv_t[:], start=True, stop=True)
                kv_v = kvps.rearrange("a (q t) -> a q t", t=256)
                out_v = ekkv_t[:, r * 1024:(r + 1) * 1024].rearrange(
                    "a (q t) -> a q t", t=256)
                if ACT_ON:
                    nc.scalar.activation(out_v[:, :, 0:128], kv_v[:, :, 0:128], AF.Exp)
                if DVE_ON:
                    nc.vector.tensor_mul(out_v[:, :, 128:256],
                                         out_v[:, :, 0:128].bitcast(F32),
                                         kv_v[:, :, 128:256])

            # ---------- q -> sigq' ----------
            sig_t = small.tile([PCH, 256], F32, tag="sig")
            qps = psB.tile([PCH, 256], F32, tag="sm")
            if PE_ON:
                for pc in range(2):
                    qsl = qps[:, pc * 128:(pc + 1) * 128]
                    nc.tensor.matmul(qsl, egT_t[:, pc * PCH:(pc + 1) * PCH],
                                     wq_t[:], start=True, stop=False)
                    nc.tensor.matmul(qsl, cap_t[:, pc * PCH:(pc + 1) * PCH],
                                     wql_t[:], start=False, stop=True)
            eq_t = small.tile([PCH, 256], F32, tag="eq")
            if ACT_ON:
                nc.scalar.activation(eq_t[:], qps[:], AF.Exp, scale=-1.0)
            if DVE_ON:
                sp_t = small.tile([PCH, 256], F32, tag="sp")
                nc.vector.tensor_scalar(sp_t[:], eq_t[:], 1.0, sdc2_ap,
                                        ALU.add, ALU.mult)
                nc.vector.reciprocal_approx_fast(sig_t[:], sp_t[:])

            # ---------- bias/denom -> aftT ----------
            aftT_t = small.tile([128, P], F32R, tag="aftT")
            eaT_v = eaT_t.rearrange("a (c p) -> a c p", p=P)
            for pc in range(2):
                bdps = psB.tile([PCH, 256], F32, tag="sm")
                if PE_ON:
                    for c in range(NCH):
                        nc.tensor.matmul(bdps[:],
                                         eaT_v[:, c, pc * PCH:(pc + 1) * PCH],
                                         ekkv_t[:, c * 256:(c + 1) * 256],
                                         start=(c == 0), stop=(c == NCH - 1))
                aft_t = small.tile([PCH, 128], F32, tag="aft")
                if DVE_ON:
                    rd_t = small.tile([PCH, 128], F32, tag="rd")
                    nc.vector.reciprocal_approx_fast(rd_t[:], bdps[:, 0:128])
                    wt_t = small.tile([PCH, 128], F32, tag="wt")
                    nc.vector.tensor_mul(wt_t[:], bdps[:, 128:256], rd_t[:])
                    nc.vector.tensor_mul(aft_t[:], wt_t[:],
                                         sig_t[:, pc * 128:(pc + 1) * 128])
                trps = psB.tile([128, PCH], F32, tag="sm")
                if PE_ON:
                    nc.tensor.transpose(trps[:], aft_t[:], ident[0:PCH, 0:PCH])
                if DVE_ON:
                    nc.vector.tensor_copy(aftT_t[:, pc * PCH:(pc + 1) * PCH], trps[:])

            # ---------- score + softmax ----------
            for pc in range(2):
                cu16_v = c2b_t[0:PCH, NCH * P + pc * N:NCH * P + (pc + 1) * N]
                curn_t = work.tile([PCH, N], F32R, tag="curn", bufs=2)
                if DVE_ON:
                    nc.gpsimd.tensor_scalar(curn_t[:], cu16_v, cst_t[0:PCH, 3:4],
                                            cst_t[0:PCH, 4:5], ALU.mult, ALU.add)
                if has_mask:
                    mkn_t = work.tile([PCH, N], F32, tag="mkn", bufs=3)
                    if DMA_ON:
                        nc.gpsimd.dma_start(mkn_t[:],
                                            mask_d[j, pc * PCH:(pc + 1) * PCH, :])
                    else:
                        nc.gpsimd.dma_start(mkn_t[:, 0:16],
                                            mask_d[j, pc * PCH:(pc + 1) * PCH, 0:16])
                th_t = work.tile([PCH, N], F32, tag="th")
                # bank-aligned score blocks; cur folded into psum via identity matmul
                for b0, bw in ((0, 1024), (1024, 976)):
                    sps = psA.tile([PCH, bw], F32, tag="big")
                    if PE_ON:
                        for o0 in range(0, bw, 512):
                            w = min(512, bw - o0)
                            nc.tensor.matmul(sps[:, o0:o0 + w],
                                             aftT_t[:, pc * PCH:(pc + 1) * PCH],
                                             enT_t[:, b0 + o0:b0 + o0 + w],
                                             start=True, stop=False)
                            nc.tensor.matmul(sps[:, o0:o0 + w],
                                             ident_r[0:PCH, 0:PCH],
                                             curn_t[:, b0 + o0:b0 + o0 + w],
                                             start=False, stop=True)
                    if ACT_ON:
                        nc.scalar.activation(th_t[:, b0:b0 + bw], sps[:], AF.Tanh,
                                             scale=c2_ap)
                e_t = work.tile([PCH, N], F32, tag="et")
                rs_t = small.tile([PCH, 1], F32, tag="rs")
                if has_mask:
                    u_t = work.tile([PCH, N], F32, tag="ut")
                    if DVE_ON:
                        nc.vector.tensor_scalar_mul(u_t[:], th_t[:], CLIP)
                        nc.vector.tensor_add(u_t[:], u_t[:], mkn_t[:])
                    if ACT_ON:
                        nc.scalar.activation(e_t[:], u_t[:], AF.Exp, accum_out=rs_t[:])
                else:
                    if ACT_ON:
                        nc.scalar.activation(e_t[:], th_t[:], AF.Exp, scale=CLIP,
                                             accum_out=rs_t[:])
                if DVE_ON:
                    rr_t = small.tile([PCH, 1], F32, tag="rr")
                    nc.vector.reciprocal(rr_t[:], rs_t[:])
                    nc.vector.tensor_scalar_mul(e_t[:], e_t[:], rr_t[:])
                st_eng = nc.sync if 'store_sp' in variant else (
                    nc.scalar if 'store_act' in variant else nc.gpsimd)
                if DMA_ON:
                    st_eng.dma_start(out_d[j, pc * PCH:(pc + 1) * PCH, :], e_t[:])
                else:
                    st_eng.dma_start(out_d[j, pc * PCH:(pc + 1) * PCH, 0:16],
                                     e_t[:, 0:16])

    nc.compile()
    return nc


def get_compiled(has_mask: bool, repeat: int = 1, variant: str = 'full'):
    key = ("k", has_mask, repeat, variant)
    if key not in _CACHE:
        _CACHE[key] = _build(has_mask, repeat, variant)
    return _CACHE[key]


def prep_inputs(inputs):
    """Host-side shard + layout prep. Returns (in_maps, has_mask)."""
    eg = np.asarray(inputs["encoded_graph_mean_pomo"], np.float32)   # [B,P,D]
    cap = np.asarray(inputs["capacity"], np.float32)                 # [B,P]
    cur = np.ascontiguousarray(np.asarray(inputs["cur_dist"], np.float32))  # [B,P,N]
    ls = float(np.asarray(inputs["log_scale"]).reshape(-1)[0])
    mask = np.asarray(inputs["ninf_mask"], np.float32)               # [B,P,N]
    en = np.asarray(inputs["encoded_nodes"], np.float32)             # [B,N,D]
    wq = np.asarray(inputs["Wq_last"], np.float32)                   # [D,D+1]
    wk = np.asarray(inputs["Wk"], np.float32)                        # [D,D]
    wv = np.asarray(inputs["Wv"], np.float32)                        # [D,D]
    a1 = float(np.asarray(inputs["AFT_dist_alpha"]).reshape(-1)[0])
    a2 = float(np.asarray(inputs["probs_dist_alpha"]).reshape(-1)[0])

    c1 = ls * a1
    c2 = ls * a2
    has_mask = bool(np.any(mask)) or (c2 == 0.0)

    if has_mask:
        # prescaled general path: A = c1*cur + mask (goes inside exp, transposed),
        # S = c2*cur (added to raw score before tanh), mask re-added after clip.
        curT_src = c1 * cur + mask
        cur_nat = c2 * cur
        sc_ea, sc_th = 1.0, 1.0
        mul2 = SQRT_D          # sigq' = sigmoid(q)/sqrt(D)
    else:
        curT_src = cur
        cur_nat = cur
        sc_ea, sc_th = c1, c2
        mul2 = SQRT_D * c2     # sigq' = sigmoid(q)/(sqrt(D)*c2)

    import ml_dtypes
    BIGW = N + P + P
    big = np.zeros((B, 128, BIGW), np.float32)
    big[:, :, 0:N] = en.transpose(0, 2, 1)                               # enT
    big[:, :, N:N + P] = eg.transpose(0, 2, 1)                           # egT
    big[:, 0, N + P:N + 2 * P] = cap                                     # cap row
    # curT: per-batch packed transpose, bf16: tile[k, c*P+p] = cur[b, p, c*CHK+k]
    curh = np.ascontiguousarray(
        curT_src.reshape(B, P, NCH, CHK).transpose(0, 3, 2, 1)
    ).reshape(B, CHK, NCH * P).astype(ml_dtypes.bfloat16)

    # curn: uint16 fixed point of cur_nat over [lo, hi]
    lo = float(cur_nat.min())
    hi = float(cur_nat.max())
    if not np.isfinite(lo) or not np.isfinite(hi) or hi <= lo:
        lo = lo if np.isfinite(lo) else 0.0
        hi = lo + 1.0
    cq = ((cur_nat - lo) * (65535.0 / (hi - lo))).round().astype(np.uint16)

    C2W = NCH * P + 2 * N
    cu2 = np.zeros((B, 128, C2W), np.uint16)
    cu2[:, 0:CHK, 0:NCH * P] = curh.view(np.uint16)
    cu2[:, 0:PCH, NCH * P:NCH * P + N] = cq[:, 0:PCH, :]
    cu2[:, 0:PCH, NCH * P + N:NCH * P + 2 * N] = cq[:, PCH:P, :]

    wkv = np.ascontiguousarray(np.concatenate([wk.T, wv.T], axis=1))  # [D,256]
    wq_m = np.ascontiguousarray(wq[:, :D].T)                 # [D,D]
    wql = np.ascontiguousarray(wq[:, D:D + 1].T)             # [1,D]
    cst = np.zeros((128, 5), np.float32)
    cst[:, 0] = sc_ea
    cst[:, 1] = sc_th
    cst[:, 2] = mul2
    cst[:, 3] = (hi - lo) / 65535.0
    cst[:, 4] = lo

    in_maps = []
    for c in range(N_CORES):
        s = slice(c * BPC, (c + 1) * BPC)
        m = {
            "big": big[s],
            "cu2": cu2[s],
            "wkv": wkv,
            "wq": wq_m,
            "wql": wql,
            "cst": cst,
        }
        if has_mask:
            m["maskn"] = np.ascontiguousarray(mask[s])
        in_maps.append(m)
    return in_maps, has_mask


def kernel(**inputs) -> np.ndarray:
    from concourse.bass_utils import run_bass_kernel_spmd
    in_maps, has_mask = prep_inputs(inputs)
    nc = get_compiled(has_mask)
    res = run_bass_kernel_spmd(nc, in_maps, core_ids=list(range(N_CORES)))
    out = np.empty((B, P, N), np.float32)
    for c in range(N_CORES):
        out[c * BPC:(c + 1) * BPC] = res.results[c]["out"]
    return out
